# revision 21
# baseline (speedup 1.0000x reference)
"""Trainium2 Bass kernel for nn_DecoderLayer (gnn_message_passing).

Sharding: flatten B*N = 4096 nodes, shard 512 nodes per core across 8 cores.
Device layout is feature-on-partition (transposed); h_E is pre-transposed on
host so the big stream DMAs straight into matmul moving operands.

Math per node n, neighbor k (reference):
  h_EV = [h_V[n], h_E[n,k]]                                (128+384)
  h1 = gelu(h_EV @ W1.T + b1); h2 = gelu(h1 @ W2.T + b2)
  msg = h2 @ W3.T + b3
  dh  = sum_k mask_attend[n,k] * msg / 30
  h   = LN1(h_V + dh)
  dh2 = gelu(h @ Win.T + bin) @ Wout.T + bout
  out = mask_V[n] * LN2(h + dh2)

Key folds:
  - W1 split: W1 = [W1V | W1E]; the W1V part uses a stride-0 broadcast AP so
    h_V is never expanded across K.
  - masked K-sum moved before W3 (linear commutes): m2[n] = sum_k mask*h2;
    dh = m2 @ (W3/30).T + (sum_k mask) * b3/30.
"""

from contextlib import ExitStack

import numpy as np

import concourse.bass as bass
import concourse.bacc as bacc
import concourse.tile as tile
from concourse import mybir
from concourse.bass_utils import run_bass_kernel_spmd

F32 = mybir.dt.float32
F32R = mybir.dt.float32r
BF16 = mybir.dt.bfloat16
AF = mybir.ActivationFunctionType
ALU = mybir.AluOpType
AX = mybir.AxisListType

H = 128
NIN = 384
FF = 512
NCHUNK = NIN // 128  # 3
FCHUNK = FF // 128   # 4
K = 48
SCALE = 30.0
EPS = 1e-5
NCORES = 8

TT = 384            # matmul tile: rows per inner tile (8 nodes * 48)
NPT = TT // K       # 8 nodes per inner tile
DMA_GROUP = 4       # inner tiles per DMA load
G_ROWS = TT * DMA_GROUP  # 1536 rows (2.36 MB) per load


GELU = AF.Gelu  # swapped out by the CoreSim test (sim lacks Gelu)


def _emit(nc, io, npc):
    """Emit the per-core program. npc = nodes per core."""
    rows = npc * K
    nt = rows // TT
    ngrp = nt // DMA_GROUP
    nblk = npc // 128
    assert rows % (TT * DMA_GROUP) == 0 and npc % 128 == 0

    with tile.TileContext(nc) as tc, ExitStack() as ctx:
        cpool = ctx.enter_context(tc.tile_pool(name="const", bufs=1))
        small = ctx.enter_context(tc.tile_pool(name="small", bufs=4))
        hpool = ctx.enter_context(tc.tile_pool(name="he", bufs=3))
        mpool = ctx.enter_context(tc.tile_pool(name="mrow", bufs=3))
        wpool = ctx.enter_context(tc.tile_pool(name="work", bufs=3))

        # ---- constants / weights (loaded once) ----
        def cload(name, shape, src_ap, dt=F32):
            t = cpool.tile(shape, dt, tag=name)
            nc.scalar.dma_start(t[:], src_ap)
            return t

        w1et = cload("w1et", [128, NCHUNK * 128],
                     io["w1et"][:].transpose([1, 0, 2]), F32R)  # (in, c, out)
        w1vt = cload("w1vt", [128, 128], io["w1vt"][:], F32R)
        w2t = cload("w2t", [128, 128], io["w2t"][:], BF16)
        w3t = cload("w3t", [128, 128], io["w3t"][:])
        wint = cload("wint", [128, FF], io["wint"][:], BF16)
        woutt = cload("woutt", [128, FCHUNK * 128],
                      io["woutt"][:].transpose([1, 0, 2]), BF16)  # (in, c, out)
        b1 = cload("b1", [128, 1], io["b1"][:])
        b2 = cload("b2", [128, 1], io["b2"][:])
        b3rep = cload("b3rep", [128, 128], io["b3rep"][:])
        binp = cload("binp", [128, FCHUNK], io["binp"][:])
        bout = cload("bout", [128, 1], io["bout"][:])
        g1rep = cload("g1rep", [128, 128], io["g1rep"][:])
        b1rep = cload("b1rep", [128, 128], io["b1rep"][:])
        g2rep = cload("g2rep", [128, 128], io["g2rep"][:])
        b2rep = cload("b2rep", [128, 128], io["b2rep"][:])
        ident = cload("ident", [128, 128], io["ident"][:])
        hv_t = cload("hv_t", [128, npc], io["hv_t"][:], F32R)
        hv_nat = cload("hv_nat", [128, nblk * 128],
                       io["hv_nat"][:].rearrange("(b p) f -> p b f", p=128))
        mask_nat = cload("mask_nat", [128, nblk * K],
                         io["mask_nat"][:].rearrange("(b p) k -> p b k", p=128))
        maskv = cload("maskv", [128, nblk], io["maskv_nat"][:])

        epsv = cpool.tile([128, 1], F32, tag="epsv")
        nc.gpsimd.memset(epsv[:], EPS)

        m2 = cpool.tile([128, npc], F32, tag="m2")
        s_mask = cpool.tile([128, nblk], F32, tag="s_mask")
        nc.vector.tensor_reduce(
            s_mask[:], mask_nat[:].rearrange("p (b k) -> p b k", k=K),
            AX.X, ALU.add)

        # All consts are loaded; rendezvous so later matmuls never carry
        # more than one DMA-sem wait (fp32/f32r matmul LDW allows only 1).
        tc.strict_bb_all_engine_barrier()

        # ---- main loop over the h_E stream ----
        h_et = io["h_et"][:]  # [NCHUNK, 128, rows]
        mask_flat = io["mask_flat"][:]  # [1, rows]
        PAIR = 2 * TT
        with tc.tile_pool(name="p1", bufs=2, space="PSUM") as p1, \
                tc.tile_pool(name="p2", bufs=2, space="PSUM") as p2:
            for g in range(ngrp):
                r0 = g * G_ROWS
                he = hpool.tile([128, NCHUNK * G_ROWS], F32R, tag="he")
                # src (p, c, r) enumeration to match dest free layout (c, r)
                nc.sync.dma_start(
                    he[:],
                    h_et[:, :, r0:r0 + G_ROWS].transpose([1, 0, 2]))
                mrow = mpool.tile([1, G_ROWS], F32, tag="mrow")
                nc.scalar.dma_start(mrow[:], mask_flat[0:1, r0:r0 + G_ROWS])
                mask_rep = mpool.tile([128, G_ROWS], F32, tag="mask_rep")
                nc.gpsimd.partition_broadcast(mask_rep[:], mrow[0:1, :])

                for q in range(DMA_GROUP // 2):
                    t0 = g * DMA_GROUP + 2 * q
                    # pair of TT-tiles; halves at 512-col (bank) offsets
                    ps1 = p1.tile([128, 1024], F32, tag="ps1")
                    for hf in range(2):
                        t = t0 + hf
                        s = 2 * q + hf
                        o = 512 * hf
                        hv_rep = hv_t[:, t * NPT:(t + 1) * NPT].unsqueeze(2) \
                            .broadcast_to([128, NPT, K])
                        nc.tensor.matmul(ps1[:, o:o + TT], w1vt[:], hv_rep,
                                         start=True, stop=False)
                        for c in range(NCHUNK):
                            nc.tensor.matmul(
                                ps1[:, o:o + TT],
                                w1et[:, c * 128:(c + 1) * 128],
                                he[:, c * G_ROWS + s * TT:
                                   c * G_ROWS + (s + 1) * TT],
                                start=False, stop=(c == NCHUNK - 1))
                    g1 = wpool.tile([128, PAIR], BF16, tag="g1")
                    ps1v = ps1[:].rearrange("p (hh c) -> p hh c", hh=2)
                    nc.scalar.activation(g1[:], ps1v[:, :, 0:TT], GELU,
                                         bias=b1[:, 0:1])

                    ps2 = p2.tile([128, 1024], F32, tag="ps2")
                    for hf in range(2):
                        o = 512 * hf
                        nc.tensor.matmul(ps2[:, o:o + TT], w2t[:],
                                         g1[:, hf * TT:(hf + 1) * TT],
                                         start=True, stop=True)
                    h2 = wpool.tile([128, PAIR], F32, tag="h2")
                    ps2v = ps2[:].rearrange("p (hh c) -> p hh c", hh=2)
                    nc.scalar.activation(h2[:], ps2v[:, :, 0:TT], GELU,
                                         bias=b2[:, 0:1])

                    tt_ = wpool.tile([128, PAIR], F32, tag="tt")
                    nc.vector.tensor_tensor(
                        tt_[:], h2[:],
                        mask_rep[:, 2 * q * TT:(2 * q + 2) * TT], ALU.mult)
                    nc.vector.tensor_reduce(
                        m2[:, t0 * NPT:(t0 + 2) * NPT],
                        tt_[:].rearrange("p (n k) -> p n k", k=K),
                        AX.X, ALU.add)

        pp = ctx.enter_context(tc.tile_pool(name="pp", bufs=4, space="PSUM"))

        # ---- message aggregation -> dh, LN1 ----
        ps_dh = pp.tile([128, npc], F32, tag="pp")
        nc.tensor.matmul(ps_dh[:], w3t[:], m2[:], start=True, stop=True)
        dh_t = cpool.tile([128, npc], F32, tag="dh_t")
        nc.scalar.activation(dh_t[:], ps_dh[:], AF.Copy)

        h_nat = cpool.tile([128, nblk * 128], F32, tag="h_nat")
        ht2 = cpool.tile([128, npc], BF16, tag="ht2")

        def layer_norm(dst, x, grep, brep, pfx):
            mu = small.tile([128, 1], F32, tag=pfx + "mu")
            nc.vector.tensor_reduce(mu[:], x[:], AX.X, ALU.add)
            nc.vector.tensor_scalar_mul(mu[:], mu[:], 1.0 / 128.0)
            nc.vector.tensor_scalar_sub(x[:], x[:], mu[:, 0:1])
            sq = wpool.tile([128, 128], F32, tag=pfx + "sq")
            var = small.tile([128, 1], F32, tag=pfx + "var")
            nc.scalar.activation(sq[:], x[:], AF.Square, accum_out=var[:])
            std = small.tile([128, 1], F32, tag=pfx + "std")
            nc.scalar.activation(std[:], var[:], AF.Sqrt,
                                 bias=epsv[:, 0:1], scale=1.0 / 128.0)
            rstd = small.tile([128, 1], F32, tag=pfx + "rstd")
            nc.vector.reciprocal(rstd[:], std[:])
            nc.vector.tensor_scalar_mul(x[:], x[:], rstd[:, 0:1])
            nc.vector.tensor_tensor(dst, x[:], grep[:], ALU.mult)
            nc.vector.tensor_tensor(dst, dst, brep[:], ALU.add)

        for j in range(nblk):
            pn = pp.tile([128, 128], F32, tag="pp")
            nc.tensor.transpose(pn[:], dh_t[:, j * 128:(j + 1) * 128], ident[:])
            x = wpool.tile([128, 128], F32, tag="x1")
            tmp = wpool.tile([128, 128], F32, tag="tmp1")
            nc.vector.tensor_scalar_mul(tmp[:], b3rep[:], s_mask[:, j:j + 1])
            nc.vector.tensor_tensor(x[:], pn[:],
                                    hv_nat[:, j * 128:(j + 1) * 128], ALU.add)
            nc.vector.tensor_tensor(x[:], x[:], tmp[:], ALU.add)
            h_slice = h_nat[:, j * 128:(j + 1) * 128]
            layer_norm(h_slice, x, g1rep, b1rep, "ln1")
            pt = pp.tile([128, 128], F32, tag="pp")
            nc.tensor.transpose(pt[:], h_slice, ident[:])
            nc.scalar.activation(ht2[:, j * 128:(j + 1) * 128], pt[:], AF.Copy)

        # ---- FFN ----
        ffr = cpool.tile([128, FCHUNK * npc], BF16, tag="ffr")
        for jo in range(FCHUNK):
            pf = pp.tile([128, npc], F32, tag="pp")
            nc.tensor.matmul(pf[:], wint[:, jo * 128:(jo + 1) * 128],
                             ht2[:], start=True, stop=True)
            nc.scalar.activation(ffr[:, jo * npc:(jo + 1) * npc], pf[:],
                                 GELU, bias=binp[:, jo:jo + 1])
        ps_dh2 = pp.tile([128, npc], F32, tag="pp")
        for jf in range(FCHUNK):
            nc.tensor.matmul(ps_dh2[:], woutt[:, jf * 128:(jf + 1) * 128],
                             ffr[:, jf * npc:(jf + 1) * npc],
                             start=(jf == 0), stop=(jf == FCHUNK - 1))
        dh2 = cpool.tile([128, npc], F32, tag="dh2")
        nc.scalar.activation(dh2[:], ps_dh2[:], AF.Identity, bias=bout[:, 0:1])

        # ---- residual 2, LN2, mask_V, store ----
        out_sb = cpool.tile([128, nblk * 128], F32, tag="out_sb")
        for j in range(nblk):
            pn = pp.tile([128, 128], F32, tag="pp")
            nc.tensor.transpose(pn[:], dh2[:, j * 128:(j + 1) * 128], ident[:])
            x = wpool.tile([128, 128], F32, tag="x2")
            nc.vector.tensor_tensor(x[:], pn[:],
                                    h_nat[:, j * 128:(j + 1) * 128], ALU.add)
            y = wpool.tile([128, 128], F32, tag="y2")
            layer_norm(y[:], x, g2rep, b2rep, "ln2")
            nc.vector.tensor_scalar_mul(out_sb[:, j * 128:(j + 1) * 128],
                                        y[:], maskv[:, j:j + 1])
        nc.scalar.dma_start(
            io["out"][:].rearrange("(b p) f -> p b f", p=128), out_sb[:])


def build_nc(npc):
    rows = npc * K
    nblk = npc // 128
    nc = bacc.Bacc()
    io = {}

    F32R_INPUTS = {"h_et", "hv_t", "w1et", "w1vt"}
    BF16_INPUTS = {"w2t", "wint", "woutt"}

    def inp(name, shape):
        dt = (F32R if name in F32R_INPUTS
              else BF16 if name in BF16_INPUTS else F32)
        io[name] = nc.dram_tensor(name, shape, dt, kind="ExternalInput")

    inp("h_et", [NCHUNK, 128, rows])
    inp("hv_t", [128, npc])
    inp("hv_nat", [npc, H])
    inp("mask_flat", [1, rows])
    inp("mask_nat", [npc, K])
    inp("maskv_nat", [128, nblk])
    inp("w1et", [NCHUNK, 128, 128])
    inp("w1vt", [128, 128])
    inp("w2t", [128, 128])
    inp("w3t", [128, 128])
    inp("wint", [128, FF])
    inp("woutt", [FCHUNK, 128, 128])
    inp("b1", [128, 1])
    inp("b2", [128, 1])
    inp("b3rep", [128, 128])
    inp("binp", [128, FCHUNK])
    inp("bout", [128, 1])
    inp("g1rep", [128, 128])
    inp("b1rep", [128, 128])
    inp("g2rep", [128, 128])
    inp("b2rep", [128, 128])
    inp("ident", [128, 128])
    io["out"] = nc.dram_tensor("out", [npc, H], F32, kind="ExternalOutput")
    _emit(nc, io, npc)
    return nc


def prep_maps(h_V, h_E, mask_V, mask_attend,
              W1_w, W1_b, W2_w, W2_b, W3_w, W3_b,
              ln1_g, ln1_b, ln2_g, ln2_b,
              Win_w, Win_b, Wout_w, Wout_b, ncores):
    f32 = np.float32
    B, N, Kk, _ = h_E.shape
    nodes = B * N
    npc = nodes // ncores
    rows = npc * Kk
    nblk = npc // 128

    hE = np.asarray(h_E, f32).reshape(ncores, npc, Kk, NIN)
    h_et = np.ascontiguousarray(hE.transpose(0, 3, 1, 2)).reshape(
        ncores, NCHUNK, 128, rows)
    hv = np.asarray(h_V, f32).reshape(ncores, npc, H)
    hv_t = np.ascontiguousarray(hv.transpose(0, 2, 1))
    mA = np.asarray(mask_attend, f32).reshape(ncores, npc, Kk)
    mV = np.asarray(mask_V, f32).reshape(ncores, nblk, 128)
    maskv_nat = np.ascontiguousarray(mV.transpose(0, 2, 1))

    import ml_dtypes
    bf16 = ml_dtypes.bfloat16

    def t(x, dt=f32):
        return np.ascontiguousarray(np.asarray(x, f32).T.astype(dt))

    rep = lambda v: np.tile(np.asarray(v, f32).reshape(1, -1), (128, 1))
    shared = {
        "w1et": np.ascontiguousarray(
            np.asarray(W1_w, f32)[:, H:].T.reshape(NCHUNK, 128, 128)),
        "w1vt": t(np.asarray(W1_w, f32)[:, :H]),
        "w2t": t(W2_w, bf16),
        "w3t": t(np.asarray(W3_w, f32) / SCALE),
        "wint": t(Win_w, bf16),
        "woutt": np.ascontiguousarray(
            np.asarray(Wout_w, f32).T.reshape(FCHUNK, 128, 128)
            .astype(bf16)),
        "b1": np.asarray(W1_b, f32).reshape(128, 1),
        "b2": np.asarray(W2_b, f32).reshape(128, 1),
        "b3rep": rep(np.asarray(W3_b, f32) / SCALE),
        "binp": np.ascontiguousarray(
            np.asarray(Win_b, f32).reshape(FCHUNK, 128).T),
        "bout": np.asarray(Wout_b, f32).reshape(128, 1),
        "g1rep": rep(ln1_g),
        "b1rep": rep(ln1_b),
        "g2rep": rep(ln2_g),
        "b2rep": rep(ln2_b),
        "ident": np.eye(128, dtype=f32),
    }
    in_maps = []
    for c in range(ncores):
        m = dict(shared)
        m["h_et"] = h_et[c]
        m["hv_t"] = hv_t[c]
        m["hv_nat"] = np.ascontiguousarray(hv[c])
        m["mask_flat"] = np.ascontiguousarray(mA[c].reshape(1, rows))
        m["mask_nat"] = np.ascontiguousarray(mA[c])
        m["maskv_nat"] = maskv_nat[c]
        in_maps.append(m)
    return in_maps, npc


_NC_CACHE = {}


def _get_nc(npc):
    if npc not in _NC_CACHE:
        nc = build_nc(npc)
        nc.finalize()
        _NC_CACHE[npc] = nc
    return _NC_CACHE[npc]


def run(inputs, trace=False):
    B, N, _, _ = inputs["h_E"].shape
    in_maps, npc = prep_maps(ncores=NCORES, **inputs)
    nc = _get_nc(npc)
    res = run_bass_kernel_spmd(nc, in_maps, core_ids=list(range(NCORES)),
                               trace=trace)
    out = np.concatenate([res.results[c]["out"] for c in range(NCORES)],
                         axis=0).reshape(B, N, H).astype(np.float32)
    return out, res.exec_time_ns


def kernel(**inputs) -> np.ndarray:
    out, _ = run(inputs)
    return out
